# revision 1
# baseline (speedup 1.0000x reference)
"""Trainium2 Bass kernel for StyleGAN2-style upsampling ConvLayer.

Reference computation (per image):
  y = conv_transpose2d(x, (w*WSCALE), stride=2)      # 512ch 64x64 -> 256ch 129x129
  y = upfirdn2d(y, fir([1,3,3,1]), pad=1, gain=4)    # 4x4 blur   -> 128x128
  y = clamp(lrelu(y + bias, 0.2) * sqrt(2), +-256)

The transposed conv + FIR chain folds exactly into 4 "parity" 3x3 SAME
convolutions of x with effective kernels W_eff = conv2d(w*WSCALE, fir*4)
subsampled per output-pixel parity:
  out[img, oc, 2P+s, 2Q+t] = sum_{ic,m,n} W_eff[oc,ic,s+2m,t+2n] * xpad[ic,P+2-m,Q+2-n]

Sharding: data parallel, 2 images per core across 8 NeuronCores.
On-chip: channel-contraction matmuls on the PE (fp32r, 1 cyc/row), PSUM
accumulation over 36 matmuls (4 ic chunks x 9 taps), ACT-engine Prelu
epilogue writing parity-interleaved staging, DVE clamp, contiguous DMA out.
"""

import numpy as np

N_CORES = 8
IMG_PER_CORE = 2
IN_CH, OUT_CH, K, UP = 512, 256, 3, 2
H = W = 64
WSCALE = float(1.0 / np.sqrt(K * K * IN_CH))
ACT_GAIN = float(np.sqrt(2.0))
CLAMP = 256.0
ALPHA = 0.2
R = 8            # parity rows per block -> matmul N = R*64 = 512
N_RB = H // R    # 8 row blocks
N_ICC = IN_CH // 128   # 4 ic chunks
N_OCC = OUT_CH // 128  # 2 oc chunks

_CACHE = {}


def _fold_weights(weight: np.ndarray) -> np.ndarray:
    """W_eff = true-conv2d(w*WSCALE, fir*gain): (256,512,6,6) in float64."""
    f1 = np.array([1.0, 3.0, 3.0, 1.0], np.float64)
    f2 = np.outer(f1, f1)
    f4 = f2 / f2.sum() * (UP * UP)
    w = weight.astype(np.float64) * WSCALE
    W_eff = np.zeros((OUT_CH, IN_CH, 6, 6), np.float64)
    for a in range(3):
        for b in range(3):
            W_eff[:, :, a:a + 4, b:b + 4] += w[:, :, a:a + 1, b:b + 1] * f4[None, None]
    return W_eff


def _prep_wp(weight: np.ndarray) -> np.ndarray:
    """wp[occhunk, ic, icchunk, parity(2s+t), tap(3m+n), oc] float32."""
    W_eff = _fold_weights(weight)
    arr = W_eff.reshape(N_OCC, 128, N_ICC, 128, 6, 6)  # [oa, o, c, i, u, v]
    wp = np.empty((N_OCC, 128, N_ICC, 4, 9, 128), np.float32)
    for s in range(2):
        for t in range(2):
            for m in range(3):
                for n in range(3):
                    blk = arr[:, :, :, :, s + 2 * m, t + 2 * n]  # [oa, o, c, i]
                    wp[:, :, :, 2 * s + t, 3 * m + n, :] = blk.transpose(0, 3, 2, 1)
    return np.ascontiguousarray(wp)


def _build_nc(n_img: int, n_rb: int):
    import concourse.bacc as bacc
    import concourse.mybir as mybir
    import concourse.tile as tile

    f32 = mybir.dt.float32
    f32r = mybir.dt.float32r

    nc = bacc.Bacc()
    xp_ext = nc.declare_dram_parameter(
        "xp", [n_img, N_ICC, 128, H + 2, W + 2], f32, isOutput=False)
    wp_ext = nc.declare_dram_parameter(
        "wp", [N_OCC, 128, N_ICC, 4, 9, 128], f32, isOutput=False)
    bg_ext = nc.declare_dram_parameter("bg", [128, N_OCC], f32, isOutput=False)
    out_ext = nc.declare_dram_parameter(
        "out", [n_img, OUT_CH, 2 * H, 2 * W], f32, isOutput=True)

    with tile.TileContext(nc) as tc:
        with (
            tc.tile_pool(name="wpool", bufs=1) as wpool,
            tc.tile_pool(name="xpool", bufs=2) as xpool,
            tc.tile_pool(name="spool", bufs=2) as spool,
            tc.tile_pool(name="bpool", bufs=1) as bpool,
            tc.tile_pool(name="ppool", bufs=8, space="PSUM") as ppool,
        ):
            bt = bpool.tile([128, N_OCC], f32)
            nc.sync.dma_start(out=bt[:], in_=bg_ext[:])

            for oa in range(N_OCC):
                wt = wpool.tile([128, N_ICC * 4 * 9 * 128], f32r, tag="wt")
                nc.sync.dma_start(out=wt[:], in_=wp_ext[oa].bitcast(f32r))
                for img in range(n_img):
                    for rb in range(n_rb):
                        r0 = rb * R
                        xts = []
                        for c in range(N_ICC):
                            xt = xpool.tile([128, R + 2, W + 2], f32r, tag=f"x{c}")
                            nc.sync.dma_start(
                                out=xt[:],
                                in_=xp_ext[img, c, :, r0:r0 + R + 2, :].bitcast(f32r))
                            xts.append(xt)
                        st = spool.tile([128, 2 * R * 2 * W], f32)
                        for pa in range(4):
                            s, t = pa >> 1, pa & 1
                            ps = ppool.tile([128, R * W], f32)
                            j = 0
                            for c in range(N_ICC):
                                for m in range(3):
                                    for n in range(3):
                                        idx = (c * 4 + pa) * 9 + (3 * m + n)
                                        rhs = xts[c][:, 2 - m:2 - m + R,
                                                     2 - n:2 - n + W]
                                        nc.tensor.matmul(
                                            ps[:],
                                            wt[:, idx * 128:(idx + 1) * 128],
                                            rhs,
                                            start=(j == 0), stop=(j == 35))
                                        j += 1
                            dst = st[:].rearrange(
                                "p (r s q t) -> p s t r q", r=R, s=2, q=W, t=2)[:, s, t]
                            nc.scalar.activation(
                                dst, ps[:].rearrange("p (r q) -> p r q", r=R),
                                mybir.ActivationFunctionType.Prelu,
                                bias=bt[:, oa:oa + 1], scale=ACT_GAIN, alpha=ALPHA)
                        nc.vector.tensor_scalar(
                            st[:], st[:], CLAMP, -CLAMP,
                            mybir.AluOpType.min, mybir.AluOpType.max)
                        nc.sync.dma_start(
                            out=out_ext[img, oa * 128:(oa + 1) * 128,
                                        2 * r0:2 * r0 + 2 * R, :],
                            in_=st[:])
    nc.compile()
    return nc


def _get_nc(n_img: int, n_rb: int):
    key = (n_img, n_rb)
    if key not in _CACHE:
        _CACHE[key] = _build_nc(n_img, n_rb)
    return _CACHE[key]


def kernel(x: np.ndarray, weight: np.ndarray, bias: np.ndarray) -> np.ndarray:
    from concourse.bass_utils import run_bass_kernel_spmd

    x = np.asarray(x, np.float32)
    weight = np.asarray(weight, np.float32)
    bias = np.asarray(bias, np.float32)

    wp = _prep_wp(weight)
    bg = np.ascontiguousarray(
        (bias.astype(np.float64) * ACT_GAIN).astype(np.float32)
        .reshape(N_OCC, 128).T)

    n_total = x.shape[0]
    xq = x.reshape(n_total, N_ICC, 128, H, W)
    xpad = np.zeros((n_total, N_ICC, 128, H + 2, W + 2), np.float32)
    xpad[:, :, :, 1:H + 1, 1:W + 1] = xq

    nc = _get_nc(IMG_PER_CORE, N_RB)
    in_maps = []
    for c in range(N_CORES):
        sl = np.ascontiguousarray(xpad[c * IMG_PER_CORE:(c + 1) * IMG_PER_CORE])
        in_maps.append({"xp": sl, "wp": wp, "bg": bg})
    res = run_bass_kernel_spmd(nc, in_maps, list(range(N_CORES)))
    out = np.concatenate([res.results[c]["out"] for c in range(N_CORES)], axis=0)
    return out
